# revision 1
# baseline (speedup 1.0000x reference)
"""Trainium2 Bass kernel for spatial self-attention (nn_Attention_90615220011343).

Module math (per batch b):
    qkv = x @ w_qkv            x:[N=4096, C=256], w_qkv:[256, 384]
    q,k,v -> heads (4 heads, dim 32)
    sim = (q*ds^-0.5) @ k^T    per head: [4096, 4096]
    attn = softmax(sim, -1)
    out = attn @ v             -> [N, 128]
    y = out @ w_out + b_out    -> [N, 256]

Sharding: 8 cores = 4 batches x 2 head-pairs. Core c -> batch c//2,
heads {2*(c%2), 2*(c%2)+1}. Each core computes a partial y (its two
heads' contribution); host sums the pair and adds b_out.

Per-core kernel layout strategy (all on-chip, no collectives):
  - x^T [2x128, 4096] via PE transposes (contraction dim C on partitions).
  - q^T replicated 4x along partitions (via host-tiled wq columns) so the
    K=32 sim matmuls can be packed 4-per-PE-pass with row tiling.
  - k^T stored "interleaved-stacked": j-chunk c (128 tokens) lives at
    partition base 32*(c%4), column block c//4. Any 3-4 consecutive
    chunks occupy distinct row-groups -> one row-tiled matmul group.
  - sim^T computed in [j, i] layout (j on partitions) so softmax exp is a
    pure elementwise pass (values are N(0,1); max-subtraction skipped -
    exp never overflows) and attn@v needs no transpose.
  - attn@v: lhsT = [v | 1] (ones column rides along, M=33) so row 32 of
    the psum accumulator is the softmax denominator for free.
  - normalization folded to the very end: y_h = (out_h @ w_out_h) scaled
    per-partition by 1/den_h, summed over the core's 2 heads on DVE.
"""

import numpy as np

HEADS = 4
DH = 32
N = 4096
C = 256
P = 128
NCH = 32  # number of 128-token j-chunks
ITILES = 8  # i tiles of 512
GROUPS = [4, 3, 4, 3, 4, 3, 4, 3, 4]  # j-chunks per sim/exp group (A/B slabs)

_CACHED = {}


def _build_nc():
    import concourse.bass as bass
    import concourse.mybir as mybir
    from concourse.tile import TileContext
    from concourse.masks import make_identity

    FP = mybir.dt.float32
    FR = mybir.dt.float32r
    AF = mybir.ActivationFunctionType
    ALU = mybir.AluOpType

    nc = bass.Bass(target_bir_lowering=False)
    x_d = nc.declare_dram_parameter("x", [N, C], FP, isOutput=False)
    wq_d = nc.declare_dram_parameter("wq", [C, 2 * P], FP, isOutput=False)
    wk_d = nc.declare_dram_parameter("wk", [C, 64], FP, isOutput=False)
    wv_d = nc.declare_dram_parameter("wv", [C, 64], FP, isOutput=False)
    wo_d = nc.declare_dram_parameter("wo", [64, C], FP, isOutput=False)
    y_d = nc.declare_dram_parameter("y", [N, C], FP, isOutput=True)

    with TileContext(nc) as tc:
        with (
            tc.tile_pool(name="const", bufs=1) as constp,
            tc.tile_pool(name="xin", bufs=10) as xinp,
            tc.tile_pool(name="big", bufs=1) as bigp,
            tc.tile_pool(name="exp", bufs=2) as expp,
            tc.tile_pool(name="ytmp", bufs=4) as ytmpp,
            tc.tile_pool(name="psA", bufs=1, space="PSUM") as psA,
            tc.tile_pool(name="psB", bufs=1, space="PSUM") as psB,
            tc.tile_pool(name="psV", bufs=1, space="PSUM") as psV,
        ):
            ident = constp.tile([P, P], FP, tag="ident")
            make_identity(nc, ident[:])

            # ---- persistent SBUF tensors ----
            xT = [bigp.tile([P, N], FR, tag=f"xT{cc}", name=f"xT{cc}") for cc in range(2)]
            qrep = [bigp.tile([P, N], FR, tag=f"qrep{h}", name=f"qrep{h}") for h in range(2)]
            karr = [bigp.tile([P, N // 4], FR, tag=f"karr{h}", name=f"karr{h}") for h in range(2)]
            vaug = [bigp.tile([P, 33 * NCH], FR, tag=f"vaug{h}", name=f"vaug{h}") for h in range(2)]
            outT = bigp.tile([64, N], FR, tag="outT")
            # softmax denominators: head h at partition 32*h
            denrow = bigp.tile([33, N], FP, tag="denrow")
            rden = bigp.tile([P, 64], FP, tag="rden")
            wq_sb = bigp.tile([P, 2, 2 * P], FR, tag="wq")
            wk_sb = bigp.tile([P, 2, 64], FR, tag="wk")
            wv_sb = bigp.tile([P, 2, 64], FR, tag="wv")
            wo_sb = bigp.tile([64, C], FR, tag="wo")

            # ---- weight + x loads (stage fp32, round to fp32r on DVE) ----
            wq_st = bigp.tile([P, 2, 2 * P], FP, tag="wq_st")
            wk_st = bigp.tile([P, 2, 64], FP, tag="wk_st")
            wv_st = bigp.tile([P, 2, 64], FP, tag="wv_st")
            wo_st = bigp.tile([64, C], FP, tag="wo_st")
            for cc in range(2):
                nc.sync.dma_start(out=wq_st[:, cc, :], in_=wq_d[cc * P:(cc + 1) * P, :])
                nc.sync.dma_start(out=wk_st[:, cc, :], in_=wk_d[cc * P:(cc + 1) * P, :])
                nc.sync.dma_start(out=wv_st[:, cc, :], in_=wv_d[cc * P:(cc + 1) * P, :])
            nc.sync.dma_start(out=wo_st[:], in_=wo_d[:])
            nc.vector.tensor_copy(out=wq_sb[:], in_=wq_st[:])
            nc.vector.tensor_copy(out=wk_sb[:], in_=wk_st[:])
            nc.vector.tensor_copy(out=wv_sb[:], in_=wv_st[:])
            nc.vector.tensor_copy(out=wo_sb[:], in_=wo_st[:])

            # ---- x load + transpose to xT ----
            # rounds: (pool, tag, nk list); each slab holds both c-chunks of
            # its nk's interleaved: [nk0/cc0, nk0/cc1, nk1/cc0, ...]
            tp_rounds = [
                (psA, "A", list(range(0, 8))),
                (psB, "B", list(range(8, 14))),
                (psA, "A", list(range(14, 22))),
                (psB, "B", list(range(22, 28))),
                (psA, "A", list(range(28, 32))),
            ]
            for pool, tag, nks in tp_rounds:
                L = 2048 if tag == "A" else 1536
                slab = pool.tile([P, L], FP, tag=tag)
                for i, nk in enumerate(nks):
                    xt = xinp.tile([P, C], FP, tag="xt")
                    dmae = nc.sync if nk % 2 == 0 else nc.scalar
                    dmae.dma_start(out=xt[:], in_=x_d[P * nk:P * (nk + 1), :])
                    for cc in range(2):
                        nc.tensor.transpose(
                            slab[:, 256 * i + P * cc: 256 * i + P * (cc + 1)],
                            xt[:, P * cc:P * (cc + 1)],
                            ident[:],
                        )
                n = len(nks)
                sv = slab[:].rearrange("p (k c f) -> p k c f", c=2, f=P)
                for cc in range(2):
                    nc.vector.tensor_copy(
                        out=xT[cc][:, P * nks[0]: P * (nks[0] + n)],
                        in_=sv[:, 0:n, cc, :],
                    )

            # ---- qkv projections ----
            def qrep_rounds(h):
                for pool, tag, it0, nits in (
                    (psA, "A", 0, 4), (psB, "B", 4, 3), (psA, "A", 7, 1),
                ):
                    L = 2048 if tag == "A" else 1536
                    slab = pool.tile([P, L], FP, tag=tag)
                    for cc in range(2):
                        for r in range(nits):
                            it = it0 + r
                            nc.tensor.matmul(
                                slab[:, 512 * r: 512 * (r + 1)],
                                lhsT=wq_sb[:, cc, P * h: P * (h + 1)],
                                rhs=xT[cc][:, 512 * it: 512 * (it + 1)],
                                start=(cc == 0), stop=(cc == 1),
                            )
                    nc.vector.tensor_copy(
                        out=qrep[h][:, 512 * it0: 512 * (it0 + nits)],
                        in_=slab[:, : 512 * nits],
                    )

            def karr_build(h):
                # karr[32*(c%4) : +32, 128*(c//4) : +128] = k^T of j-chunk c
                # (col-tiling is incompatible with fp32r: all matmuls write
                # partition base 0; DVE relocates to the stacked layout)
                for p_ in range(2):
                    slab = psA.tile([P, 2048], FP, tag="A")
                    for ct in range(4):
                        # rhs: j-chunks c = 4m+ct for m in [4p, 4p+4) -> strided view
                        for cc in range(2):
                            xv = xT[cc][:].rearrange(
                                "q (m t f) -> q m t f", t=4, f=P
                            )[:, 4 * p_: 4 * p_ + 4, ct, :]
                            nc.tensor.matmul(
                                slab[0:32, 512 * ct: 512 * (ct + 1)],
                                lhsT=wk_sb[:, cc, 32 * h: 32 * (h + 1)],
                                rhs=xv,
                                start=(cc == 0), stop=(cc == 1),
                            )
                    for ct in range(4):
                        nc.vector.tensor_copy(
                            out=karr[h][32 * ct: 32 * (ct + 1), 512 * p_: 512 * (p_ + 1)],
                            in_=slab[0:32, 512 * ct: 512 * (ct + 1)],
                        )

            def v_build():
                # both heads at once: psum [128, 64*(k%..)] chunks
                slab = psA.tile([P, 2048], FP, tag="A")
                for k in range(NCH):
                    for cc in range(2):
                        nc.tensor.matmul(
                            slab[:, 64 * k: 64 * (k + 1)],
                            lhsT=xT[cc][:, P * k: P * (k + 1)],
                            rhs=wv_sb[:, cc, :],
                            start=(cc == 0), stop=(cc == 1),
                        )
                sv = slab[:].rearrange("p (k d) -> p k d", d=64)
                ones_st = bigp.tile([P, NCH], FP, tag="ones_st")
                nc.gpsimd.memset(ones_st[:], 1.0)
                for h in range(2):
                    vv = vaug[h][:].rearrange("p (k e) -> p k e", e=33)
                    nc.vector.tensor_copy(out=vv[:, :, 32], in_=ones_st[:])
                    nc.vector.tensor_copy(
                        out=vv[:, :, 0:32], in_=sv[:, :, 32 * h: 32 * (h + 1)]
                    )

            qrep_rounds(0)
            karr_build(0)
            v_build()

            # head-0 projection accumulator (filled during head-1 attention)
            yacc = bigp.tile([P, NCH * C], FP, tag="yacc")
            yv = yacc[:].rearrange("p (k c) -> p k c", c=C)

            def attention(h, post_it=None):
                vv = vaug[h][:].rearrange("p (k e) -> p k e", e=33)
                for it in range(ITILES):
                    i0 = 512 * it
                    av = psV.tile([P, 512], FP, tag="V")
                    cstart = 0
                    for gsz in GROUPS:
                        pool, tag, L = (psA, "A", 2048) if gsz == 4 else (psB, "B", 1536)
                        slab = pool.tile([P, L], FP, tag=tag)
                        for r in range(gsz):
                            c = cstart + r
                            rt = c % 4
                            nc.tensor.matmul(
                                slab[:, 512 * r: 512 * (r + 1)],
                                lhsT=karr[h][32 * rt: 32 * (rt + 1), P * (c // 4): P * (c // 4 + 1)],
                                rhs=qrep[h][32 * rt: 32 * (rt + 1), i0: i0 + 512],
                                start=True, stop=True,
                                tile_position=(32 * rt, 0),
                            )
                        eslab = expp.tile([P, L], FR, tag="E")
                        nc.scalar.activation(eslab[:], slab[:], AF.Exp)
                        for r in range(gsz):
                            c = cstart + r
                            nc.tensor.matmul(
                                av[0:33, :],
                                lhsT=vv[:, c, :],
                                rhs=eslab[:, 512 * r: 512 * (r + 1)],
                                start=(c == 0), stop=(c == NCH - 1),
                                skip_group_check=True,
                            )
                        cstart += gsz
                    nc.vector.tensor_copy(out=outT[32 * h: 32 * h + 32, i0: i0 + 512], in_=av[0:32, :])
                    nc.vector.tensor_copy(out=denrow[32 * h: 32 * h + 1, i0: i0 + 512], in_=av[32:33, :])
                    if post_it is not None:
                        post_it(it)

            def den_recip(h):
                # denominator row -> column layout, reciprocal
                dslab = psV.tile([P, 512], FP, tag="V")
                for t in range(NCH):
                    nc.tensor.transpose(
                        dslab[:, t: t + 1],
                        denrow[32 * h: 32 * h + 1, P * t: P * (t + 1)],
                        ident[32 * h: 32 * h + 1, 32 * h: 32 * h + 1],
                    )
                nc.vector.reciprocal(out=rden[:, 32 * h: 32 * h + 32], in_=dslab[:, 0:32])

            def y0_chunks(it):
                # head-0 output projection, interleaved into head-1 attention
                for k in range(4 * it, 4 * it + 4):
                    yp = psV.tile([P, 512], FP, tag="V")
                    nc.tensor.matmul(
                        yp[:, 0:C], lhsT=outT[0:32, P * k: P * (k + 1)],
                        rhs=wo_sb[0:32, :],
                        start=True, stop=True, tile_position=(0, 0),
                    )
                    nc.vector.tensor_scalar_mul(yv[:, k, :], yp[:, 0:C], rden[:, k: k + 1])

            attention(0)
            den_recip(0)
            qrep_rounds(1)
            karr_build(1)
            attention(1, post_it=y0_chunks)
            den_recip(1)

            # ---- tail: head-1 projection + combine + store ----
            for k in range(NCH):
                pool, tag, L = (psA, "A", 2048) if k % 2 == 0 else (psB, "B", 1536)
                yb = pool.tile([P, L], FP, tag=tag)
                nc.tensor.matmul(
                    yb[:, 0:C], lhsT=outT[32:64, P * k: P * (k + 1)],
                    rhs=wo_sb[32:64, :],
                    start=True, stop=True, tile_position=(32, 0),
                )
                yo = ytmpp.tile([P, C], FP, tag="yo")
                nc.vector.scalar_tensor_tensor(
                    out=yo[:], in0=yb[:, 0:C], scalar=rden[:, 32 + k: 33 + k],
                    in1=yv[:, k, :], op0=ALU.mult, op1=ALU.add,
                )
                dmae = nc.sync if k % 2 == 0 else nc.scalar
                dmae.dma_start(out=y_d[P * k: P * (k + 1), :], in_=yo[:])

    _split_excess_waits(nc, mybir)
    return nc


def _split_excess_waits(nc, mybir, maxw=1, carrier_cap=1):
    """walrus codegen allows few semaphore waits per engine instruction.

    Tile's scheduler can emit 3-4 on one matmul. Hoist the excess onto
    InstEventSemaphore carriers inserted immediately before the instruction
    on the same engine queue (queue is FIFO, so waiting in the carrier is
    equivalent; no reordering so no deadlock risk).
    """
    skip = {
        "InstEventSemaphore", "InstCall",
        "InstUnconditionalBranch", "InstISA", "InstRegisterMove",
    }
    for f in nc.m.functions:
        for blk in f.blocks:
            idx = 0
            while idx < len(blk.instructions):
                ins = blk.instructions[idx]
                si = getattr(ins, "sync_info", None)
                if (
                    si is not None and si.on_wait and len(si.on_wait) > maxw
                    and type(ins).__name__ not in skip
                ):
                    waits = list(si.on_wait)
                    keep, excess = waits[:maxw], waits[maxw:]
                    n_ins = 0
                    for i in range(0, len(excess), carrier_cap):
                        ev = mybir.InstEventSemaphore(
                            name=nc.get_next_instruction_name(),
                            engine=ins.engine,
                            ins=[], outs=[],
                            sync_info=mybir.SyncInfo(
                                on_wait=excess[i:i + carrier_cap], on_update=[]
                            ),
                        )
                        nc.register_instruction(ev)
                        blk.instructions.insert(idx + n_ins, ev)
                        n_ins += 1
                    ins.sync_info = mybir.SyncInfo(
                        on_wait=keep, on_update=list(si.on_update or [])
                    )
                    idx += n_ins
                idx += 1
    return nc


def get_nc():
    if "nc" not in _CACHED:
        _CACHED["nc"] = _build_nc()
    return _CACHED["nc"]


def make_in_maps(x, w_qkv, w_out):
    """Host-side sharding: core c -> batch c//2, heads (c%2)*2, (c%2)*2+1."""
    B = x.shape[0]
    xf = np.ascontiguousarray(x.reshape(B, N, C))
    scale = DH ** -0.5
    in_maps = []
    for core in range(8):
        b, hp = core // 2, core % 2
        h0, h1 = 2 * hp, 2 * hp + 1
        wq = np.concatenate(
            [np.tile(w_qkv[:, h * DH:(h + 1) * DH] * scale, (1, 4)) for h in (h0, h1)],
            axis=1,
        )  # [256, 256]
        wk = np.concatenate(
            [w_qkv[:, 128 + h * DH: 128 + (h + 1) * DH] for h in (h0, h1)], axis=1
        )  # [256, 64]
        wv = np.concatenate(
            [w_qkv[:, 256 + h * DH: 256 + (h + 1) * DH] for h in (h0, h1)], axis=1
        )  # [256, 64]
        wo = np.concatenate(
            [w_out[h * DH:(h + 1) * DH, :] for h in (h0, h1)], axis=0
        )  # [64, 256]
        in_maps.append({
            "x": np.ascontiguousarray(xf[b]),
            "wq": np.ascontiguousarray(wq.astype(np.float32)),
            "wk": np.ascontiguousarray(wk.astype(np.float32)),
            "wv": np.ascontiguousarray(wv.astype(np.float32)),
            "wo": np.ascontiguousarray(wo.astype(np.float32)),
        })
    return in_maps


def kernel(x, w_qkv, w_out, b_out):
    from concourse.bass_utils import run_bass_kernel_spmd

    nc = get_nc()
    in_maps = make_in_maps(
        np.asarray(x, dtype=np.float32),
        np.asarray(w_qkv, dtype=np.float32),
        np.asarray(w_out, dtype=np.float32),
    )
    res = run_bass_kernel_spmd(nc, in_maps, list(range(8))).results
    B, H, W = 4, 64, 64
    y = np.empty((B, N, C), dtype=np.float32)
    for b in range(B):
        y[b] = res[2 * b]["y"] + res[2 * b + 1]["y"]
    y += np.asarray(b_out, dtype=np.float32)
    return y.reshape(B, H, W, C)



# revision 8
# speedup vs baseline: 1.1998x; 1.1998x over previous
"""Trainium2 Bass kernel for spatial self-attention (nn_Attention_90615220011343).

Module math (per batch b):
    qkv = x @ w_qkv            x:[N=4096, C=256], w_qkv:[256, 384]
    q,k,v -> heads (4 heads, dim 32)
    sim = (q*ds^-0.5) @ k^T    per head: [4096, 4096]
    attn = softmax(sim, -1)
    out = attn @ v             -> [N, 128]
    y = out @ w_out + b_out    -> [N, 256]

Sharding: 8 cores = 4 batches x 2 head-pairs. Core c -> batch c//2,
heads {2*(c%2), 2*(c%2)+1}. Each core computes a partial y (its two
heads' contribution); host sums the pair and adds b_out.

v3 design (bf16 datapath, dual-engine softmax):
  - Host ships x pre-transposed and bf16-quantized (xtb [128, 2cc, 4096]),
    weights as bf16 slices.  All big matmuls run bf16 at 1 PE-cycle/column
    (vs fp32's 4 and fp32r's 256-col constraint); sim contraction d=32,
    both heads packed at partition bases {0, 32}.
  - The Schraudolph scale 128*log2(e) is folded into wq host-side, so
    sim_meas = 128*log2e * sim_true.  Softmax max-subtraction is skipped
    (bf16 exp range is huge); exp splits across two engines per psum slab:
      ACT: native Exp (scale=1/A16) -> bf16
      DVE: one tensor_scalar_add: bits16 = round(sim_meas + B16) written
           as uint16 into the bf16 tile (Schraudolph exp; HW-verified
           round-to-nearest saturating convert)
  - attn@v: lhsT = [v | ones] (M=33) so psum row 32 accumulates the
    softmax denominator for free; out^T lands d-major for the y-proj.
  - y: per-head fp32r projection of out^T, scaled by 1/den during psum
    evacuation (ACT Copy*scale for head 0, DVE scalar_tensor_tensor for
    head 1), streamed to DRAM per 128-token chunk.
"""

import numpy as np

HEADS = 4
DH = 32
N = 4096
C = 256
P = 128
NCH = 32   # 128-token j-chunks
ITILES = 8
GROUPS = [4, 3, 4, 3, 4, 3, 4, 3, 4]  # sim/exp chunks per psum slab (A/B)

LOG2E = 1.4426950408889634
A16 = 128.0 * LOG2E              # sim_meas = A16 * sim_true
F16 = A16 / np.sqrt(32.0)        # folded into wq host-side
C16 = 7.2                        # schraudolph mantissa-bias tuning
B16 = 128.0 * 127.0 - C16
# ACT's share of each exp slab, in columns (rest goes to DVE schraudolph)
SPLIT = {4: 1152, 3: 864}

_CACHED = {}


def _build_nc():
    import concourse.bass as bass
    import concourse.mybir as mybir
    from concourse.tile import TileContext
    from concourse.masks import make_identity

    FP = mybir.dt.float32
    FR = mybir.dt.float32r
    BF = mybir.dt.bfloat16
    U16 = mybir.dt.uint16
    AF = mybir.ActivationFunctionType
    ALU = mybir.AluOpType

    nc = bass.Bass(target_bir_lowering=False)
    x_d = nc.declare_dram_parameter("xtb", [P, 2, N], BF, isOutput=False)
    wqk_d = nc.declare_dram_parameter("wqk", [P, 2, 2, 64], BF, isOutput=False)
    wv_d = nc.declare_dram_parameter("wvb", [P, 2, 64], BF, isOutput=False)
    wo_d = nc.declare_dram_parameter("wo", [64, C], FP, isOutput=False)
    y_d = nc.declare_dram_parameter("y", [N, C], FP, isOutput=True)

    with TileContext(nc) as tc:
        with (
            tc.tile_pool(name="const", bufs=1) as constp,
            tc.tile_pool(name="big", bufs=1) as bigp,
            tc.tile_pool(name="exp", bufs=2) as expp,
            tc.tile_pool(name="ytmp", bufs=4) as ytmpp,
            tc.tile_pool(name="psA", bufs=1, space="PSUM") as psA,
            tc.tile_pool(name="psB", bufs=1, space="PSUM") as psB,
            tc.tile_pool(name="psV", bufs=1, space="PSUM") as psV,
        ):
            ident = constp.tile([P, P], FP, tag="ident")
            make_identity(nc, ident[:])
            bias0 = constp.tile([P, 1], FP, tag="bias0")
            nc.gpsimd.memset(bias0[:], 0.0)

            # ---- persistent SBUF ----
            xtb = bigp.tile([P, 2, N], BF, tag="xtb")
            qbQ = bigp.tile([64, ITILES, 512], BF, tag="qbQ")
            qbK = bigp.tile([64, NCH, P], BF, tag="qbK")
            vbf = [bigp.tile([P, NCH, 33], BF, tag=f"vbf{h}", name=f"vbf{h}")
                   for h in range(2)]
            wqk = bigp.tile([P, 2, 2, 64], BF, tag="wqk")
            wvb = bigp.tile([P, 2, 64], BF, tag="wvb")
            wo_st = bigp.tile([64, C], FP, tag="wo_st")
            wo_sb = [bigp.tile([32, C], FR, tag=f"wo{h}", name=f"wo{h}")
                     for h in range(2)]
            outd = [bigp.tile([33, N], FR, tag=f"outd{h}", name=f"outd{h}")
                    for h in range(2)]
            rden = bigp.tile([P, 64], FP, tag="rden")
            yacc = bigp.tile([P, NCH, C], BF, tag="yacc")

            # ---- loads ----
            nc.sync.dma_start(out=wqk[:], in_=wqk_d[:])
            nc.sync.dma_start(out=wvb[:], in_=wv_d[:])
            nc.sync.dma_start(out=wo_st[:], in_=wo_d[:])
            nc.vector.tensor_copy(out=wo_sb[0][:], in_=wo_st[0:32, :])
            nc.vector.tensor_copy(out=wo_sb[1][:], in_=wo_st[32:64, :])
            for e in range(8):
                nc.sync.dma_start(
                    out=xtb[:, :, 512 * e: 512 * (e + 1)],
                    in_=x_d[:, :, 512 * e: 512 * (e + 1)],
                )
            for h in range(2):
                nc.gpsimd.memset(vbf[h][:, :, 32:33], 1.0)

            # ---- Q/K projections: 8 windows of 512 ----
            # per window: q -> [0:64, cols 0:512], k -> [64:128, cols 512:1024]
            # (separate column regions so each gets its own psum zero-region)
            for w in range(ITILES):
                pool, tag = (psA, "A") if w % 2 == 0 else (psB, "B")
                L = 2048 if w % 2 == 0 else 1536
                slab = pool.tile([P, L], FP, tag=tag)
                qv = slab[0:64, 0:512]
                kv = slab[64:128, 512:1024]
                for cc in range(2):
                    nc.tensor.matmul(
                        qv, lhsT=wqk[:, cc, 0, :],
                        rhs=xtb[:, cc, 512 * w: 512 * (w + 1)],
                        start=(cc == 0), stop=(cc == 1),
                        tile_position=(0, 0),
                    )
                for cc in range(2):
                    nc.tensor.matmul(
                        kv, lhsT=wqk[:, cc, 1, :],
                        rhs=xtb[:, cc, 512 * w: 512 * (w + 1)],
                        start=(cc == 0), stop=(cc == 1),
                        tile_position=(0, 64),
                    )
                if w % 2 == 0:
                    nc.scalar.activation(qbQ[:, w, :], qv, AF.Copy)
                    nc.vector.tensor_copy(
                        out=qbK[:, 4 * w: 4 * w + 4, :],
                        in_=kv.rearrange("p (c j) -> p c j", j=P),
                    )
                else:
                    nc.vector.tensor_copy(out=qbQ[:, w, :], in_=qv)
                    nc.scalar.activation(
                        qbK[:, 4 * w: 4 * w + 4, :],
                        kv.rearrange("p (c j) -> p c j", j=P),
                        AF.Copy,
                    )

            # ---- V projection (both heads at once) ----
            vslab = psA.tile([P, 2048], FP, tag="A")
            vv = vslab[:, :].rearrange("p (c m) -> p c m", m=64)
            for c in range(NCH):
                for cc in range(2):
                    nc.tensor.matmul(
                        vv[:, c, :],
                        lhsT=xtb[:, cc, P * c: P * (c + 1)],
                        rhs=wvb[:, cc, :],
                        start=(cc == 0), stop=(cc == 1),
                    )
            nc.vector.tensor_copy(out=vbf[0][:, :, 0:32], in_=vv[:, :, 0:32])
            nc.scalar.activation(vbf[1][:, :, 0:32], vv[:, :, 32:64], AF.Copy)

            # ---- attention ----
            def attention(h, post_it=None):
                for it in range(ITILES):
                    i0 = 512 * it
                    av = psV.tile([P, 512], FP, tag="V")
                    e16 = expp.tile([P, NCH, 512], BF, tag="E")

                    def attnv(c):
                        nc.tensor.matmul(
                            av[0:33, :],
                            lhsT=vbf[h][:, c, :],
                            rhs=e16[:, c, :],
                            start=(c == 0), stop=(c == NCH - 1),
                            skip_group_check=True,
                        )

                    cstart = 0
                    adone = 0
                    prev_cstart = 0
                    for gsz in GROUPS:
                        pool, tag, L = (psA, "A", 2048) if gsz == 4 else (psB, "B", 1536)
                        slab = pool.tile([P, L], FP, tag=tag)
                        for r in range(gsz):
                            c = cstart + r
                            nc.tensor.matmul(
                                slab[:, 512 * r: 512 * (r + 1)],
                                lhsT=qbK[32 * h: 32 * h + 32, c, :],
                                rhs=qbQ[32 * h: 32 * h + 32, it, :],
                                start=True, stop=True,
                            )
                        # exp of this slab (ACT | DVE split)
                        ev = e16[:, cstart: cstart + gsz, :].rearrange("p c n -> p (c n)")
                        sl = slab[:, 0: 512 * gsz]
                        sa = SPLIT[gsz]
                        nc.scalar.activation(
                            ev[:, 0:sa], sl[:, 0:sa], AF.Exp,
                            bias=bias0[:], scale=1.0 / A16,
                        )
                        nc.vector.tensor_scalar_add(
                            ev[:, sa:].bitcast(U16), sl[:, sa:], B16
                        )
                        # attn@v lagged one slab behind
                        while adone < prev_cstart:
                            attnv(adone)
                            adone += 1
                        prev_cstart = cstart + gsz
                        cstart += gsz
                    while adone < NCH:
                        attnv(adone)
                        adone += 1
                    if it % 2 == 0:
                        nc.vector.tensor_copy(
                            out=outd[h][:, i0: i0 + 512], in_=av[0:33, :]
                        )
                    else:
                        nc.scalar.activation(
                            outd[h][:, i0: i0 + 512], av[0:33, :], AF.Copy
                        )
                    if post_it is not None:
                        post_it(it)

            def den_recip(h):
                dslab = psV.tile([P, 512], FP, tag="V")
                for t in range(NCH):
                    nc.tensor.transpose(
                        dslab[:, t: t + 1],
                        outd[h][32:33, P * t: P * (t + 1)].bitcast(FP),
                        ident[32:33, 32:33],
                    )
                nc.vector.reciprocal(
                    out=rden[:, 32 * h: 32 * h + 32], in_=dslab[:, 0:32]
                )

            def y0_chunks(it):
                for k in range(4 * it, 4 * it + 4):
                    yp = psV.tile([P, 512], FP, tag="V")
                    nc.tensor.matmul(
                        yp[:, 0:C],
                        lhsT=outd[0][0:32, P * k: P * (k + 1)],
                        rhs=wo_sb[0][:],
                        start=True, stop=True,
                    )
                    nc.scalar.activation(
                        yacc[:, k, :], yp[:, 0:C], AF.Copy,
                        scale=rden[:, k: k + 1],
                    )

            attention(0)
            den_recip(0)
            attention(1, post_it=y0_chunks)
            den_recip(1)

            # ---- tail: head-1 projection + combine + store ----
            for k in range(NCH):
                pool, tag, L = (psA, "A", 2048) if k % 2 == 0 else (psB, "B", 1536)
                yb = pool.tile([P, L], FP, tag=tag)
                nc.tensor.matmul(
                    yb[:, 0:C],
                    lhsT=outd[1][0:32, P * k: P * (k + 1)],
                    rhs=wo_sb[1][:],
                    start=True, stop=True,
                )
                yo = ytmpp.tile([P, C], FP, tag="yo")
                nc.vector.scalar_tensor_tensor(
                    out=yo[:], in0=yb[:, 0:C], scalar=rden[:, 32 + k: 33 + k],
                    in1=yacc[:, k, :], op0=ALU.mult, op1=ALU.add,
                )
                nc.sync.dma_start(out=y_d[P * k: P * (k + 1), :], in_=yo[:])

    _split_excess_waits(nc, mybir)
    return nc


def _split_excess_waits(nc, mybir, maxw=1, carrier_cap=1):
    """walrus codegen allows few semaphore waits per engine instruction.

    Tile's scheduler can emit 3-4 on one matmul. Hoist the excess onto
    InstEventSemaphore carriers inserted immediately before the instruction
    on the same engine queue (queue is FIFO, so waiting in the carrier is
    equivalent; no reordering so no deadlock risk).
    """
    skip = {
        "InstEventSemaphore", "InstCall",
        "InstUnconditionalBranch", "InstISA", "InstRegisterMove",
    }
    for f in nc.m.functions:
        for blk in f.blocks:
            idx = 0
            while idx < len(blk.instructions):
                ins = blk.instructions[idx]
                si = getattr(ins, "sync_info", None)
                if (
                    si is not None and si.on_wait and len(si.on_wait) > maxw
                    and type(ins).__name__ not in skip
                ):
                    waits = list(si.on_wait)
                    keep, excess = waits[:maxw], waits[maxw:]
                    n_ins = 0
                    for i in range(0, len(excess), carrier_cap):
                        ev = mybir.InstEventSemaphore(
                            name=nc.get_next_instruction_name(),
                            engine=ins.engine,
                            ins=[], outs=[],
                            sync_info=mybir.SyncInfo(
                                on_wait=excess[i:i + carrier_cap], on_update=[]
                            ),
                        )
                        nc.register_instruction(ev)
                        blk.instructions.insert(idx + n_ins, ev)
                        n_ins += 1
                    ins.sync_info = mybir.SyncInfo(
                        on_wait=keep, on_update=list(si.on_update or [])
                    )
                    idx += n_ins
                idx += 1
    return nc


def get_nc():
    if "nc" not in _CACHED:
        _CACHED["nc"] = _build_nc()
    return _CACHED["nc"]


def make_in_maps(x, w_qkv, w_out):
    """Host-side sharding: core c -> batch c//2, heads (c%2)*2, (c%2)*2+1."""
    import ml_dtypes

    bft = ml_dtypes.bfloat16
    B = x.shape[0]
    xf = np.ascontiguousarray(np.asarray(x, np.float32).reshape(B, N, C))
    wq_all = np.asarray(w_qkv, np.float32)
    wo_all = np.asarray(w_out, np.float32)
    in_maps = []
    for core in range(8):
        b, hp = core // 2, core % 2
        h0 = 2 * hp
        # xtb[p, cc, j] = x[j, cc*128+p]
        xtb = np.ascontiguousarray(
            xf[b].T.reshape(2, P, N).transpose(1, 0, 2)
        ).astype(bft)
        # wqk[p, cc, 0, hh*32+d] = wq[cc*128+p, (h0+hh)*32+d] * F16
        # wqk[p, cc, 1, hh*32+d] = wk[cc*128+p, (h0+hh)*32+d]
        qcols = wq_all[:, h0 * 32: h0 * 32 + 64] * F16          # [256, 64]
        kcols = wq_all[:, 128 + h0 * 32: 128 + h0 * 32 + 64]    # [256, 64]
        wqk = np.stack([qcols.reshape(2, P, 64), kcols.reshape(2, P, 64)],
                       axis=2)                                   # [cc, p, qk, 64]
        wqk = np.ascontiguousarray(wqk.transpose(1, 0, 2, 3)).astype(bft)
        vcols = wq_all[:, 256 + h0 * 32: 256 + h0 * 32 + 64]     # [256, 64]
        wvb = np.ascontiguousarray(
            vcols.reshape(2, P, 64).transpose(1, 0, 2)
        ).astype(bft)
        wo = np.concatenate(
            [wo_all[(h0 + hh) * DH: (h0 + hh + 1) * DH, :] for hh in range(2)],
            axis=0,
        )
        in_maps.append({
            "xtb": xtb,
            "wqk": wqk,
            "wvb": wvb,
            "wo": np.ascontiguousarray(wo.astype(np.float32)),
        })
    return in_maps


def kernel(x, w_qkv, w_out, b_out):
    from concourse.bass_utils import run_bass_kernel_spmd

    nc = get_nc()
    in_maps = make_in_maps(x, w_qkv, w_out)
    res = run_bass_kernel_spmd(nc, in_maps, list(range(8))).results
    B, H, W = 4, 64, 64
    y = np.empty((B, N, C), dtype=np.float32)
    for b in range(B):
        y[b] = res[2 * b]["y"] + res[2 * b + 1]["y"]
    y += np.asarray(b_out, dtype=np.float32)
    return y.reshape(B, H, W, C)


# revision 16
# speedup vs baseline: 1.5755x; 1.3131x over previous
"""Trainium2 Bass kernel for spatial self-attention (nn_Attention_90615220011343).

Module math (per batch b):
    qkv = x @ w_qkv            x:[N=4096, C=256], w_qkv:[256, 384]
    q,k,v -> heads (4 heads, dim 32)
    sim = (q*ds^-0.5) @ k^T    per head: [4096, 4096]
    attn = softmax(sim, -1)
    out = attn @ v             -> [N, 128]
    y = out @ w_out + b_out    -> [N, 256]

Sharding: 8 cores = 4 batches x 2 head-pairs. Core c -> batch c//2,
heads {2*(c%2), 2*(c%2)+1}. Each core computes a partial y (its two
heads' contribution); host sums the pair and adds b_out.

v3 design (bf16 datapath, dual-engine softmax):
  - Host ships x pre-transposed and bf16-quantized (xtb [128, 2cc, 4096]),
    weights as bf16 slices.  All big matmuls run bf16 at 1 PE-cycle/column
    (vs fp32's 4 and fp32r's 256-col constraint); sim contraction d=32,
    both heads packed at partition bases {0, 32}.
  - The Schraudolph scale 128*log2(e) is folded into wq host-side, so
    sim_meas = 128*log2e * sim_true.  Softmax max-subtraction is skipped
    (bf16 exp range is huge); exp splits across two engines per psum slab:
      ACT: native Exp (scale=1/A16) -> bf16
      DVE: one tensor_scalar_add: bits16 = round(sim_meas + B16) written
           as uint16 into the bf16 tile (Schraudolph exp; HW-verified
           round-to-nearest saturating convert)
  - attn@v: lhsT = [v | ones] (M=33) so psum row 32 accumulates the
    softmax denominator for free; out^T lands d-major for the y-proj.
  - y: per-head fp32r projection of out^T, scaled by 1/den during psum
    evacuation (ACT Copy*scale for head 0, DVE scalar_tensor_tensor for
    head 1), streamed to DRAM per 128-token chunk.
"""

import numpy as np

HEADS = 4
DH = 32
N = 4096
C = 256
P = 128
NCH = 32   # 128-token j-chunks
ITILES = 8
GROUPS = [2] * 16  # sim/exp chunks per psum slab (A/B/C rotation)

LOG2E = 1.4426950408889634
A16 = 128.0 * LOG2E              # sim_meas = A16 * sim_true
F16 = A16 / np.sqrt(32.0)        # folded into wq host-side
C16 = 7.2                        # schraudolph mantissa-bias tuning
B16 = 128.0 * 127.0 - C16
# ACT's share of each exp slab, in columns (rest goes to DVE schraudolph)
SPLIT = {2: 576}

_CACHED = {}


def _build_nc():
    import concourse.bass as bass
    import concourse.mybir as mybir
    from concourse.tile import TileContext
    from concourse.masks import make_identity

    FP = mybir.dt.float32
    FR = mybir.dt.float32r
    BF = mybir.dt.bfloat16
    U16 = mybir.dt.uint16
    AF = mybir.ActivationFunctionType
    ALU = mybir.AluOpType

    nc = bass.Bass(target_bir_lowering=False)
    x_d = nc.declare_dram_parameter("xtb", [P, 2, N], BF, isOutput=False)
    wqk_d = nc.declare_dram_parameter("wqk", [P, 2, 2, 64], BF, isOutput=False)
    wv_d = nc.declare_dram_parameter("wvb", [P, 2, 64], BF, isOutput=False)
    wo_d = nc.declare_dram_parameter("wo", [64, C], FP, isOutput=False)
    y_d = nc.declare_dram_parameter("y", [N, C], FP, isOutput=True)

    with TileContext(nc) as tc:
        with (
            tc.tile_pool(name="const", bufs=1) as constp,
            tc.tile_pool(name="big", bufs=1) as bigp,
            tc.tile_pool(name="exp", bufs=2) as expp,
            tc.tile_pool(name="ytmp", bufs=4) as ytmpp,
            tc.tile_pool(name="psA", bufs=1, space="PSUM") as psA,
            tc.tile_pool(name="psB", bufs=1, space="PSUM") as psB,
            tc.tile_pool(name="psC", bufs=1, space="PSUM") as psC,
            tc.tile_pool(name="psV", bufs=1, space="PSUM") as psV,
            tc.tile_pool(name="psY", bufs=1, space="PSUM") as psY,
        ):
            ident = constp.tile([P, P], FP, tag="ident")
            make_identity(nc, ident[:])
            bias0 = constp.tile([P, 1], FP, tag="bias0")
            nc.gpsimd.memset(bias0[:], 0.0)

            # ---- persistent SBUF ----
            xtb = bigp.tile([P, 2, N], BF, tag="xtb")
            qbQ = bigp.tile([64, ITILES, 512], BF, tag="qbQ")
            qbK = bigp.tile([64, NCH, P], BF, tag="qbK")
            vbf = [bigp.tile([P, NCH, 33], BF, tag=f"vbf{h}", name=f"vbf{h}")
                   for h in range(2)]
            wqk = bigp.tile([P, 2, 2, 64], BF, tag="wqk")
            wvb = bigp.tile([P, 2, 64], BF, tag="wvb")
            wo_st = bigp.tile([64, C], FP, tag="wo_st")
            wo_sb = [bigp.tile([32, C], FR, tag=f"wo{h}", name=f"wo{h}")
                     for h in range(2)]
            outd = [bigp.tile([33, N], FR, tag=f"outd{h}", name=f"outd{h}")
                    for h in range(2)]
            rden = bigp.tile([P, 64], FP, tag="rden")
            yacc = bigp.tile([P, NCH, C], BF, tag="yacc")

            # ---- loads ----
            nc.sync.dma_start(out=wqk[:], in_=wqk_d[:])
            nc.sync.dma_start(out=wvb[:], in_=wv_d[:])
            nc.sync.dma_start(out=wo_st[:], in_=wo_d[:])
            nc.vector.tensor_copy(out=wo_sb[0][:], in_=wo_st[0:32, :])
            nc.vector.tensor_copy(out=wo_sb[1][:], in_=wo_st[32:64, :])
            dmaq = [nc.sync, nc.scalar, nc.gpsimd]
            for e in range(8):
                dmaq[e % 3].dma_start(
                    out=xtb[:, :, 512 * e: 512 * (e + 1)],
                    in_=x_d[:, :, 512 * e: 512 * (e + 1)],
                )
            for h in range(2):
                nc.gpsimd.memset(vbf[h][:, :, 32:33], 1.0)

            # ---- Q/K projections: 8 windows of 512 ----
            # per window: q -> [0:64, cols 0:512], k -> [64:128, cols 512:1024]
            # (separate column regions so each gets its own psum zero-region)
            for w in range(ITILES):
                pool, tag = [(psA, "A"), (psB, "B"), (psC, "C")][w % 3]
                slab = pool.tile([P, 1024], FP, tag=tag)
                qv = slab[0:64, 0:512]
                kv = slab[64:128, 512:1024]
                for cc in range(2):
                    nc.tensor.matmul(
                        qv, lhsT=wqk[:, cc, 0, :],
                        rhs=xtb[:, cc, 512 * w: 512 * (w + 1)],
                        start=(cc == 0), stop=(cc == 1),
                        tile_position=(0, 0),
                    )
                for cc in range(2):
                    nc.tensor.matmul(
                        kv, lhsT=wqk[:, cc, 1, :],
                        rhs=xtb[:, cc, 512 * w: 512 * (w + 1)],
                        start=(cc == 0), stop=(cc == 1),
                        tile_position=(0, 64),
                    )
                if w % 2 == 0:
                    nc.scalar.activation(qbQ[:, w, :], qv, AF.Copy)
                    nc.vector.tensor_copy(
                        out=qbK[:, 4 * w: 4 * w + 4, :],
                        in_=kv.rearrange("p (c j) -> p c j", j=P),
                    )
                else:
                    nc.vector.tensor_copy(out=qbQ[:, w, :], in_=qv)
                    nc.scalar.activation(
                        qbK[:, 4 * w: 4 * w + 4, :],
                        kv.rearrange("p (c j) -> p c j", j=P),
                        AF.Copy,
                    )

            # ---- V projection (both heads at once) ----
            for half, (pool, tag) in enumerate([(psA, "A"), (psB, "B")]):
                vslab = pool.tile([P, 1024], FP, tag=tag)
                c0 = 16 * half
                vv = vslab[:, 0:1024].rearrange("p (c m) -> p c m", m=64)
                for c in range(16):
                    for cc in range(2):
                        nc.tensor.matmul(
                            vv[:, c, :],
                            lhsT=xtb[:, cc, P * (c0 + c): P * (c0 + c + 1)],
                            rhs=wvb[:, cc, :],
                            start=(cc == 0), stop=(cc == 1),
                        )
                nc.vector.tensor_copy(
                    out=vbf[0][:, c0: c0 + 16, 0:32], in_=vv[:, :, 0:32]
                )
                nc.scalar.activation(
                    vbf[1][:, c0: c0 + 16, 0:32], vv[:, :, 32:64], AF.Copy
                )

            # ---- attention ----
            def attention(h, post_it=None):
                for it in range(ITILES):
                    i0 = 512 * it
                    av = psV.tile([P, 512], FP, tag="V")
                    e16 = expp.tile([P, NCH, 512], BF, tag="E")

                    def attnv(c):
                        nc.tensor.matmul(
                            av[0:33, :],
                            lhsT=vbf[h][:, c, :],
                            rhs=e16[:, c, :],
                            start=(c == 0), stop=(c == NCH - 1),
                            skip_group_check=True,
                        )

                    cstart = 0
                    adone = 0
                    hist = [0, 0]  # cstart as of one / two groups ago
                    for gi, gsz in enumerate(GROUPS):
                        pool, tag = [(psA, "A"), (psB, "B"), (psC, "C")][gi % 3]
                        slab = pool.tile([P, 1024], FP, tag=tag)
                        for r in range(gsz):
                            c = cstart + r
                            nc.tensor.matmul(
                                slab[:, 512 * r: 512 * (r + 1)],
                                lhsT=qbK[32 * h: 32 * h + 32, c, :],
                                rhs=qbQ[32 * h: 32 * h + 32, it, :],
                                start=True, stop=True,
                            )
                        # exp of this slab (whole slabs alternate ACT / DVE)
                        ev = e16[:, cstart: cstart + gsz, :].rearrange("p c n -> p (c n)")
                        sl = slab[:, 0: 512 * gsz]
                        if gi % 2 == 0:
                            nc.scalar.activation(
                                ev, sl, AF.Exp, bias=bias0[:], scale=1.0 / A16,
                            )
                        else:
                            nc.vector.tensor_scalar_add(
                                ev.bitcast(U16), sl, B16
                            )
                        # attn@v lagged two slabs behind
                        while adone < hist[1]:
                            attnv(adone)
                            adone += 1
                        hist = [cstart + gsz, hist[0]]
                        cstart += gsz
                    while adone < NCH:
                        attnv(adone)
                        adone += 1
                    if it % 2 == 0:
                        nc.vector.tensor_copy(
                            out=outd[h][:, i0: i0 + 512], in_=av[0:33, :]
                        )
                    else:
                        nc.scalar.activation(
                            outd[h][:, i0: i0 + 512], av[0:33, :], AF.Copy
                        )
                    if post_it is not None:
                        post_it(it)

            def den_recip(h):
                dslab = psY.tile([P, 512], FP, tag="Y")
                for t in range(NCH):
                    nc.tensor.transpose(
                        dslab[:, t: t + 1],
                        outd[h][32:33, P * t: P * (t + 1)].bitcast(FP),
                        ident[32:33, 32:33],
                    )
                nc.vector.reciprocal(
                    out=rden[:, 32 * h: 32 * h + 32], in_=dslab[:, 0:32]
                )

            def y0_chunks(it):
                for k in range(4 * it, 4 * it + 4):
                    yp = psY.tile([P, 512], FP, tag="Y")
                    nc.tensor.matmul(
                        yp[:, 0:C],
                        lhsT=outd[0][0:32, P * k: P * (k + 1)],
                        rhs=wo_sb[0][:],
                        start=True, stop=True,
                    )
                    nc.scalar.activation(
                        yacc[:, k, :], yp[:, 0:C], AF.Copy,
                        scale=rden[:, k: k + 1],
                    )

            attention(0)
            den_recip(0)
            attention(1, post_it=y0_chunks)
            den_recip(1)

            # ---- tail: head-1 projection + combine + store ----
            for k in range(NCH):
                pool, tag = [(psA, "A"), (psB, "B"), (psC, "C")][k % 3]
                yb = pool.tile([P, 1024], FP, tag=tag)
                nc.tensor.matmul(
                    yb[:, 0:C],
                    lhsT=outd[1][0:32, P * k: P * (k + 1)],
                    rhs=wo_sb[1][:],
                    start=True, stop=True,
                )
                yo = ytmpp.tile([P, C], FP, tag="yo")
                nc.vector.scalar_tensor_tensor(
                    out=yo[:], in0=yb[:, 0:C], scalar=rden[:, 32 + k: 33 + k],
                    in1=yacc[:, k, :], op0=ALU.mult, op1=ALU.add,
                )
                nc.sync.dma_start(out=y_d[P * k: P * (k + 1), :], in_=yo[:])

    _split_excess_waits(nc, mybir)
    return nc


def _split_excess_waits(nc, mybir, maxw=1, carrier_cap=1):
    """walrus codegen allows few semaphore waits per engine instruction.

    Tile's scheduler can emit 3-4 on one matmul. Hoist the excess onto
    InstEventSemaphore carriers inserted immediately before the instruction
    on the same engine queue (queue is FIFO, so waiting in the carrier is
    equivalent; no reordering so no deadlock risk).
    """
    skip = {
        "InstEventSemaphore", "InstCall",
        "InstUnconditionalBranch", "InstISA", "InstRegisterMove",
    }
    for f in nc.m.functions:
        for blk in f.blocks:
            idx = 0
            while idx < len(blk.instructions):
                ins = blk.instructions[idx]
                si = getattr(ins, "sync_info", None)
                if (
                    si is not None and si.on_wait and len(si.on_wait) > maxw
                    and type(ins).__name__ not in skip
                ):
                    waits = list(si.on_wait)
                    keep, excess = waits[:maxw], waits[maxw:]
                    n_ins = 0
                    for i in range(0, len(excess), carrier_cap):
                        ev = mybir.InstEventSemaphore(
                            name=nc.get_next_instruction_name(),
                            engine=ins.engine,
                            ins=[], outs=[],
                            sync_info=mybir.SyncInfo(
                                on_wait=excess[i:i + carrier_cap], on_update=[]
                            ),
                        )
                        nc.register_instruction(ev)
                        blk.instructions.insert(idx + n_ins, ev)
                        n_ins += 1
                    ins.sync_info = mybir.SyncInfo(
                        on_wait=keep, on_update=list(si.on_update or [])
                    )
                    idx += n_ins
                idx += 1
    return nc


def get_nc():
    if "nc" not in _CACHED:
        _CACHED["nc"] = _build_nc()
    return _CACHED["nc"]


def make_in_maps(x, w_qkv, w_out):
    """Host-side sharding: core c -> batch c//2, heads (c%2)*2, (c%2)*2+1."""
    import ml_dtypes

    bft = ml_dtypes.bfloat16
    B = x.shape[0]
    xf = np.ascontiguousarray(np.asarray(x, np.float32).reshape(B, N, C))
    wq_all = np.asarray(w_qkv, np.float32)
    wo_all = np.asarray(w_out, np.float32)
    in_maps = []
    for core in range(8):
        b, hp = core // 2, core % 2
        h0 = 2 * hp
        # xtb[p, cc, j] = x[j, cc*128+p]
        xtb = np.ascontiguousarray(
            xf[b].T.reshape(2, P, N).transpose(1, 0, 2)
        ).astype(bft)
        # wqk[p, cc, 0, hh*32+d] = wq[cc*128+p, (h0+hh)*32+d] * F16
        # wqk[p, cc, 1, hh*32+d] = wk[cc*128+p, (h0+hh)*32+d]
        qcols = wq_all[:, h0 * 32: h0 * 32 + 64] * F16          # [256, 64]
        kcols = wq_all[:, 128 + h0 * 32: 128 + h0 * 32 + 64]    # [256, 64]
        wqk = np.stack([qcols.reshape(2, P, 64), kcols.reshape(2, P, 64)],
                       axis=2)                                   # [cc, p, qk, 64]
        wqk = np.ascontiguousarray(wqk.transpose(1, 0, 2, 3)).astype(bft)
        vcols = wq_all[:, 256 + h0 * 32: 256 + h0 * 32 + 64]     # [256, 64]
        wvb = np.ascontiguousarray(
            vcols.reshape(2, P, 64).transpose(1, 0, 2)
        ).astype(bft)
        wo = np.concatenate(
            [wo_all[(h0 + hh) * DH: (h0 + hh + 1) * DH, :] for hh in range(2)],
            axis=0,
        )
        in_maps.append({
            "xtb": xtb,
            "wqk": wqk,
            "wvb": wvb,
            "wo": np.ascontiguousarray(wo.astype(np.float32)),
        })
    return in_maps


def kernel(x, w_qkv, w_out, b_out):
    from concourse.bass_utils import run_bass_kernel_spmd

    nc = get_nc()
    in_maps = make_in_maps(x, w_qkv, w_out)
    res = run_bass_kernel_spmd(nc, in_maps, list(range(8))).results
    B, H, W = 4, 64, 64
    y = np.empty((B, N, C), dtype=np.float32)
    for b in range(B):
        y[b] = res[2 * b]["y"] + res[2 * b + 1]["y"]
    y += np.asarray(b_out, dtype=np.float32)
    return y.reshape(B, H, W, C)


# revision 24
# speedup vs baseline: 1.6519x; 1.0485x over previous
"""Trainium2 Bass kernel for spatial self-attention (nn_Attention_90615220011343).

Module math (per batch b):
    qkv = x @ w_qkv            x:[N=4096, C=256], w_qkv:[256, 384]
    q,k,v -> heads (4 heads, dim 32)
    sim = (q*ds^-0.5) @ k^T    per head: [4096, 4096]
    attn = softmax(sim, -1)
    out = attn @ v             -> [N, 128]
    y = out @ w_out + b_out    -> [N, 256]

Sharding: 8 cores = 4 batches x 2 head-pairs. Core c -> batch c//2,
heads {2*(c%2), 2*(c%2)+1}. Each core computes a partial y (its two
heads' contribution); host sums the pair and adds b_out.

v3 design (bf16 datapath, dual-engine softmax):
  - Host ships x pre-transposed and bf16-quantized (xtb [128, 2cc, 4096]),
    weights as bf16 slices.  All big matmuls run bf16 at 1 PE-cycle/column
    (vs fp32's 4 and fp32r's 256-col constraint); sim contraction d=32,
    both heads packed at partition bases {0, 32}.
  - The Schraudolph scale 128*log2(e) is folded into wq host-side, so
    sim_meas = 128*log2e * sim_true.  Softmax max-subtraction is skipped
    (bf16 exp range is huge); exp splits across two engines per psum slab:
      ACT: native Exp (scale=1/A16) -> bf16
      DVE: one tensor_scalar_add: bits16 = round(sim_meas + B16) written
           as uint16 into the bf16 tile (Schraudolph exp; HW-verified
           round-to-nearest saturating convert)
  - attn@v: lhsT = [v | ones] (M=33) so psum row 32 accumulates the
    softmax denominator for free; out^T lands d-major for the y-proj.
  - y: per-head fp32r projection of out^T, scaled by 1/den during psum
    evacuation (ACT Copy*scale for head 0, DVE scalar_tensor_tensor for
    head 1), streamed to DRAM per 128-token chunk.
"""

import numpy as np

HEADS = 4
DH = 32
N = 4096
C = 256
P = 128
NCH = 32   # 128-token j-chunks
ITILES = 8
GROUPS = [2] * 16  # sim/exp chunks per psum slab (A/B/C rotation)

LOG2E = 1.4426950408889634
A16 = 128.0 * LOG2E              # sim_meas = A16 * sim_true
F16 = A16 / np.sqrt(32.0)        # folded into wq host-side
C16 = 7.2                        # schraudolph mantissa-bias tuning
B16 = 128.0 * 127.0 - C16
# ACT's share of each exp slab, in columns (rest goes to DVE schraudolph)
SPLIT = {2: 576}

_CACHED = {}


def _build_nc():
    import concourse.bass as bass
    import concourse.mybir as mybir
    from concourse.tile import TileContext
    from concourse.masks import make_identity

    FP = mybir.dt.float32
    FR = mybir.dt.float32r
    BF = mybir.dt.bfloat16
    U16 = mybir.dt.uint16
    AF = mybir.ActivationFunctionType
    ALU = mybir.AluOpType

    nc = bass.Bass(target_bir_lowering=False)
    x_d = nc.declare_dram_parameter("xtb", [P, 2, N], BF, isOutput=False)
    wqk_d = nc.declare_dram_parameter("wqk", [P, 2, 2, 64], BF, isOutput=False)
    wv_d = nc.declare_dram_parameter("wvb", [P, 2, 64], BF, isOutput=False)
    wo_d = nc.declare_dram_parameter("wo", [64, C], FP, isOutput=False)
    y_d = nc.declare_dram_parameter("y", [N, C], FP, isOutput=True)

    with TileContext(nc) as tc:
        with (
            tc.tile_pool(name="const", bufs=1) as constp,
            tc.tile_pool(name="big", bufs=1) as bigp,
            tc.tile_pool(name="exp", bufs=2) as expp,
            tc.tile_pool(name="ytmp", bufs=4) as ytmpp,
            tc.tile_pool(name="psA", bufs=1, space="PSUM") as psA,
            tc.tile_pool(name="psB", bufs=1, space="PSUM") as psB,
            tc.tile_pool(name="psC", bufs=1, space="PSUM") as psC,
            tc.tile_pool(name="psV", bufs=1, space="PSUM") as psV,
            tc.tile_pool(name="psY", bufs=1, space="PSUM") as psY,
        ):
            ident = constp.tile([P, P], FP, tag="ident")
            make_identity(nc, ident[:])
            bias0 = constp.tile([P, 1], FP, tag="bias0")
            nc.gpsimd.memset(bias0[:], 0.0)

            # ---- persistent SBUF ----
            xtb = bigp.tile([P, 2, N], BF, tag="xtb")
            qbQ = bigp.tile([64, ITILES, 512], BF, tag="qbQ")
            qbK = bigp.tile([64, NCH, P], BF, tag="qbK")
            vbf = [bigp.tile([P, NCH, 33], BF, tag=f"vbf{h}", name=f"vbf{h}")
                   for h in range(2)]
            wqk = bigp.tile([P, 2, 2, 64], BF, tag="wqk")
            wvb = bigp.tile([P, 2, 64], BF, tag="wvb")
            wo_st = bigp.tile([64, C], FP, tag="wo_st")
            wo_sb = [bigp.tile([32, C], FR, tag=f"wo{h}", name=f"wo{h}")
                     for h in range(2)]
            outd = [bigp.tile([33, N], FR, tag=f"outd{h}", name=f"outd{h}")
                    for h in range(2)]
            rden = bigp.tile([P, 64], FP, tag="rden")
            yacc = bigp.tile([P, NCH, C], BF, tag="yacc")

            # ---- loads ----
            dmaq = [nc.sync, nc.scalar, nc.gpsimd]
            nc.scalar.dma_start(out=wqk[:], in_=wqk_d[:])
            for e in range(8):
                dmaq[e % 3].dma_start(
                    out=xtb[:, :, 512 * e: 512 * (e + 1)],
                    in_=x_d[:, :, 512 * e: 512 * (e + 1)],
                )
            nc.gpsimd.dma_start(out=wvb[:], in_=wv_d[:])
            nc.sync.dma_start(out=wo_st[:], in_=wo_d[:])
            nc.vector.tensor_copy(out=wo_sb[0][:], in_=wo_st[0:32, :])
            nc.vector.tensor_copy(out=wo_sb[1][:], in_=wo_st[32:64, :])
            for h in range(2):
                nc.gpsimd.memset(vbf[h][:, :, 32:33], 1.0)

            # ---- Q/K projections: 8 windows of 512 ----
            # per window: q -> [0:64, cols 0:512], k -> [64:128, cols 512:1024]
            # (separate column regions so each gets its own psum zero-region)
            for w in range(ITILES):
                pool, tag = [(psA, "A"), (psB, "B"), (psC, "C")][w % 3]
                slab = pool.tile([P, 1024], FP, tag=tag)
                qv = slab[0:64, 0:512]
                kv = slab[64:128, 512:1024]
                for cc in range(2):
                    nc.tensor.matmul(
                        qv, lhsT=wqk[:, cc, 0, :],
                        rhs=xtb[:, cc, 512 * w: 512 * (w + 1)],
                        start=(cc == 0), stop=(cc == 1),
                        tile_position=(0, 0),
                    )
                for cc in range(2):
                    nc.tensor.matmul(
                        kv, lhsT=wqk[:, cc, 1, :],
                        rhs=xtb[:, cc, 512 * w: 512 * (w + 1)],
                        start=(cc == 0), stop=(cc == 1),
                        tile_position=(0, 64),
                    )
                if w % 2 == 0:
                    nc.scalar.activation(qbQ[:, w, :], qv, AF.Copy)
                    nc.vector.tensor_copy(
                        out=qbK[:, 4 * w: 4 * w + 4, :],
                        in_=kv.rearrange("p (c j) -> p c j", j=P),
                    )
                else:
                    nc.vector.tensor_copy(out=qbQ[:, w, :], in_=qv)
                    nc.scalar.activation(
                        qbK[:, 4 * w: 4 * w + 4, :],
                        kv.rearrange("p (c j) -> p c j", j=P),
                        AF.Copy,
                    )

            # ---- V projection (both heads at once) ----
            for half, (pool, tag) in enumerate([(psA, "A"), (psB, "B")]):
                vslab = pool.tile([P, 1024], FP, tag=tag)
                c0 = 16 * half
                vv = vslab[:, 0:1024].rearrange("p (c m) -> p c m", m=64)
                for c in range(16):
                    for cc in range(2):
                        nc.tensor.matmul(
                            vv[:, c, :],
                            lhsT=xtb[:, cc, P * (c0 + c): P * (c0 + c + 1)],
                            rhs=wvb[:, cc, :],
                            start=(cc == 0), stop=(cc == 1),
                        )
                nc.vector.tensor_copy(
                    out=vbf[0][:, c0: c0 + 16, 0:32], in_=vv[:, :, 0:32]
                )
                nc.scalar.activation(
                    vbf[1][:, c0: c0 + 16, 0:32], vv[:, :, 32:64], AF.Copy
                )

            # ---- attention ----
            def attention(h, post_it=None):
                for it in range(ITILES):
                    i0 = 512 * it
                    av = psV.tile([P, 512], FP, tag="V")
                    e16 = expp.tile([P, NCH, 512], BF, tag="E")

                    def attnv(c):
                        nc.tensor.matmul(
                            av[0:33, :],
                            lhsT=vbf[h][:, c, :],
                            rhs=e16[:, c, :],
                            start=(c == 0), stop=(c == NCH - 1),
                            skip_group_check=True,
                        )

                    cstart = 0
                    adone = 0
                    hist = [0, 0, 0]  # cstart as of 1/2/3 groups ago
                    for gi, gsz in enumerate(GROUPS):
                        pool, tag = [(psA, "A"), (psB, "B"), (psC, "C")][gi % 3]
                        slab = pool.tile([P, 1024], FP, tag=tag)
                        for r in range(gsz):
                            c = cstart + r
                            nc.tensor.matmul(
                                slab[:, 512 * r: 512 * (r + 1)],
                                lhsT=qbK[32 * h: 32 * h + 32, c, :],
                                rhs=qbQ[32 * h: 32 * h + 32, it, :],
                                start=True, stop=True,
                            )
                        # exp of this slab (whole slabs alternate ACT / DVE)
                        ev = e16[:, cstart: cstart + gsz, :].rearrange("p c n -> p (c n)")
                        sl = slab[:, 0: 512 * gsz]
                        if gi % 2 == 0:
                            nc.scalar.activation(
                                ev, sl, AF.Exp, bias=bias0[:], scale=1.0 / A16,
                            )
                        else:
                            nc.vector.tensor_scalar_add(
                                ev.bitcast(U16), sl, B16
                            )
                        # attn@v lagged three slabs behind
                        while adone < hist[2]:
                            attnv(adone)
                            adone += 1
                        hist = [cstart + gsz, hist[0], hist[1]]
                        cstart += gsz
                    while adone < NCH:
                        attnv(adone)
                        adone += 1
                    if it % 2 == 0:
                        nc.vector.tensor_copy(
                            out=outd[h][:, i0: i0 + 512], in_=av[0:33, :]
                        )
                    else:
                        nc.scalar.activation(
                            outd[h][:, i0: i0 + 512], av[0:33, :], AF.Copy
                        )
                    if post_it is not None:
                        post_it(it)

            def den_recip(h):
                dslab = psY.tile([P, 512], FP, tag="Y")
                for t in range(NCH):
                    nc.tensor.transpose(
                        dslab[:, t: t + 1],
                        outd[h][32:33, P * t: P * (t + 1)].bitcast(FP),
                        ident[32:33, 32:33],
                    )
                nc.vector.reciprocal(
                    out=rden[:, 32 * h: 32 * h + 32], in_=dslab[:, 0:32]
                )

            def y_chunks(it):
                # head-0 projection into yacc (rden[0] fully ready)
                for k in range(4 * it, 4 * it + 4):
                    yp = psY.tile([P, 512], FP, tag="Y")
                    nc.tensor.matmul(
                        yp[:, 0:C],
                        lhsT=outd[0][0:32, P * k: P * (k + 1)],
                        rhs=wo_sb[0][:],
                        start=True, stop=True,
                    )
                    nc.scalar.activation(
                        yacc[:, k, :], yp[:, 0:C], AF.Copy,
                        scale=rden[:, k: k + 1],
                    )
                # head-1 denominators for this i-tile
                dslab = psY.tile([P, 512], FP, tag="Y")
                for t in range(4 * it, 4 * it + 4):
                    nc.tensor.transpose(
                        dslab[:, t - 4 * it: t - 4 * it + 1],
                        outd[1][32:33, P * t: P * (t + 1)].bitcast(FP),
                        ident[32:33, 32:33],
                    )
                nc.vector.reciprocal(
                    out=rden[:, 32 + 4 * it: 36 + 4 * it], in_=dslab[:, 0:4]
                )
                # head-1 projection + combine + store
                for k in range(4 * it, 4 * it + 4):
                    yb = psY.tile([P, 512], FP, tag="Y")
                    nc.tensor.matmul(
                        yb[:, 0:C],
                        lhsT=outd[1][0:32, P * k: P * (k + 1)],
                        rhs=wo_sb[1][:],
                        start=True, stop=True,
                    )
                    yo = ytmpp.tile([P, C], FP, tag="yo")
                    nc.vector.scalar_tensor_tensor(
                        out=yo[:], in0=yb[:, 0:C], scalar=rden[:, 32 + k: 33 + k],
                        in1=yacc[:, k, :], op0=ALU.mult, op1=ALU.add,
                    )
                    dmaq[k % 3].dma_start(out=y_d[P * k: P * (k + 1), :], in_=yo[:])

            def den0_chunks(it):
                dslab = psY.tile([P, 512], FP, tag="Y")
                for t in range(4 * it, 4 * it + 4):
                    nc.tensor.transpose(
                        dslab[:, t - 4 * it: t - 4 * it + 1],
                        outd[0][32:33, P * t: P * (t + 1)].bitcast(FP),
                        ident[32:33, 32:33],
                    )
                nc.vector.reciprocal(
                    out=rden[:, 4 * it: 4 * it + 4], in_=dslab[:, 0:4]
                )

            attention(0, post_it=den0_chunks)
            attention(1, post_it=y_chunks)

    _split_excess_waits(nc, mybir)
    return nc


def _split_excess_waits(nc, mybir, maxw=1, carrier_cap=1):
    """walrus codegen allows few semaphore waits per engine instruction.

    Tile's scheduler can emit 3-4 on one matmul. Hoist the excess onto
    InstEventSemaphore carriers inserted immediately before the instruction
    on the same engine queue (queue is FIFO, so waiting in the carrier is
    equivalent; no reordering so no deadlock risk).
    """
    skip = {
        "InstEventSemaphore", "InstCall",
        "InstUnconditionalBranch", "InstISA", "InstRegisterMove",
    }
    for f in nc.m.functions:
        for blk in f.blocks:
            idx = 0
            while idx < len(blk.instructions):
                ins = blk.instructions[idx]
                si = getattr(ins, "sync_info", None)
                if (
                    si is not None and si.on_wait and len(si.on_wait) > maxw
                    and type(ins).__name__ not in skip
                ):
                    waits = list(si.on_wait)
                    keep, excess = waits[:maxw], waits[maxw:]
                    n_ins = 0
                    for i in range(0, len(excess), carrier_cap):
                        ev = mybir.InstEventSemaphore(
                            name=nc.get_next_instruction_name(),
                            engine=ins.engine,
                            ins=[], outs=[],
                            sync_info=mybir.SyncInfo(
                                on_wait=excess[i:i + carrier_cap], on_update=[]
                            ),
                        )
                        nc.register_instruction(ev)
                        blk.instructions.insert(idx + n_ins, ev)
                        n_ins += 1
                    ins.sync_info = mybir.SyncInfo(
                        on_wait=keep, on_update=list(si.on_update or [])
                    )
                    idx += n_ins
                idx += 1
    return nc


def get_nc():
    if "nc" not in _CACHED:
        _CACHED["nc"] = _build_nc()
    return _CACHED["nc"]


def make_in_maps(x, w_qkv, w_out):
    """Host-side sharding: core c -> batch c//2, heads (c%2)*2, (c%2)*2+1."""
    import ml_dtypes

    bft = ml_dtypes.bfloat16
    B = x.shape[0]
    xf = np.ascontiguousarray(np.asarray(x, np.float32).reshape(B, N, C))
    wq_all = np.asarray(w_qkv, np.float32)
    wo_all = np.asarray(w_out, np.float32)
    in_maps = []
    for core in range(8):
        b, hp = core // 2, core % 2
        h0 = 2 * hp
        # xtb[p, cc, j] = x[j, cc*128+p]
        xtb = np.ascontiguousarray(
            xf[b].T.reshape(2, P, N).transpose(1, 0, 2)
        ).astype(bft)
        # wqk[p, cc, 0, hh*32+d] = wq[cc*128+p, (h0+hh)*32+d] * F16
        # wqk[p, cc, 1, hh*32+d] = wk[cc*128+p, (h0+hh)*32+d]
        qcols = wq_all[:, h0 * 32: h0 * 32 + 64] * F16          # [256, 64]
        kcols = wq_all[:, 128 + h0 * 32: 128 + h0 * 32 + 64]    # [256, 64]
        wqk = np.stack([qcols.reshape(2, P, 64), kcols.reshape(2, P, 64)],
                       axis=2)                                   # [cc, p, qk, 64]
        wqk = np.ascontiguousarray(wqk.transpose(1, 0, 2, 3)).astype(bft)
        vcols = wq_all[:, 256 + h0 * 32: 256 + h0 * 32 + 64]     # [256, 64]
        wvb = np.ascontiguousarray(
            vcols.reshape(2, P, 64).transpose(1, 0, 2)
        ).astype(bft)
        wo = np.concatenate(
            [wo_all[(h0 + hh) * DH: (h0 + hh + 1) * DH, :] for hh in range(2)],
            axis=0,
        )
        in_maps.append({
            "xtb": xtb,
            "wqk": wqk,
            "wvb": wvb,
            "wo": np.ascontiguousarray(wo.astype(np.float32)),
        })
    return in_maps


def kernel(x, w_qkv, w_out, b_out):
    from concourse.bass_utils import run_bass_kernel_spmd

    nc = get_nc()
    in_maps = make_in_maps(x, w_qkv, w_out)
    res = run_bass_kernel_spmd(nc, in_maps, list(range(8))).results
    B, H, W = 4, 64, 64
    y = np.empty((B, N, C), dtype=np.float32)
    for b in range(B):
        y[b] = res[2 * b]["y"] + res[2 * b + 1]["y"]
    y += np.asarray(b_out, dtype=np.float32)
    return y.reshape(B, H, W, C)


# revision 28
# speedup vs baseline: 1.6853x; 1.0202x over previous
"""Trainium2 Bass kernel for spatial self-attention (nn_Attention_90615220011343).

Module math (per batch b):
    qkv = x @ w_qkv            x:[N=4096, C=256], w_qkv:[256, 384]
    q,k,v -> heads (4 heads, dim 32)
    sim = (q*ds^-0.5) @ k^T    per head: [4096, 4096]
    attn = softmax(sim, -1)
    out = attn @ v             -> [N, 128]
    y = out @ w_out + b_out    -> [N, 256]

Sharding: 8 cores = 4 batches x 2 head-pairs. Core c -> batch c//2,
heads {2*(c%2), 2*(c%2)+1}. Each core computes a partial y (its two
heads' contribution); host sums the pair and adds b_out.

v3 design (bf16 datapath, dual-engine softmax):
  - Host ships x pre-transposed and bf16-quantized (xtb [128, 2cc, 4096]),
    weights as bf16 slices.  All big matmuls run bf16 at 1 PE-cycle/column
    (vs fp32's 4 and fp32r's 256-col constraint); sim contraction d=32,
    both heads packed at partition bases {0, 32}.
  - The Schraudolph scale 128*log2(e) is folded into wq host-side, so
    sim_meas = 128*log2e * sim_true.  Softmax max-subtraction is skipped
    (bf16 exp range is huge); exp splits across two engines per psum slab:
      ACT: native Exp (scale=1/A16) -> bf16
      DVE: one tensor_scalar_add: bits16 = round(sim_meas + B16) written
           as uint16 into the bf16 tile (Schraudolph exp; HW-verified
           round-to-nearest saturating convert)
  - attn@v: lhsT = [v | ones] (M=33) so psum row 32 accumulates the
    softmax denominator for free; out^T lands d-major for the y-proj.
  - y: per-head fp32r projection of out^T, scaled by 1/den during psum
    evacuation (ACT Copy*scale for head 0, DVE scalar_tensor_tensor for
    head 1), streamed to DRAM per 128-token chunk.
"""

import numpy as np

HEADS = 4
DH = 32
N = 4096
C = 256
P = 128
NCH = 32   # 128-token j-chunks
ITILES = 8
GROUPS = [2] * 16  # sim/exp chunks per psum slab (A/B/C rotation)

LOG2E = 1.4426950408889634
A16 = 128.0 * LOG2E              # sim_meas = A16 * sim_true
F16 = A16 / np.sqrt(32.0)        # folded into wq host-side
C16 = 7.2                        # schraudolph mantissa-bias tuning
B16 = 128.0 * 127.0 - C16
# ACT's share of each exp slab, in columns (rest goes to DVE schraudolph)
SPLIT = {2: 576}

_CACHED = {}


def _build_nc():
    import concourse.bass as bass
    import concourse.mybir as mybir
    from concourse.tile import TileContext
    from concourse.masks import make_identity

    FP = mybir.dt.float32
    FR = mybir.dt.float32r
    BF = mybir.dt.bfloat16
    U16 = mybir.dt.uint16
    AF = mybir.ActivationFunctionType
    ALU = mybir.AluOpType

    nc = bass.Bass(target_bir_lowering=False)
    x_d = nc.declare_dram_parameter("xtb", [P, 2, N], BF, isOutput=False)
    wqk_d = nc.declare_dram_parameter("wqk", [P, 2, 128], BF, isOutput=False)
    wv_d = nc.declare_dram_parameter("wvb", [P, 2, 64], BF, isOutput=False)
    wo_d = nc.declare_dram_parameter("wo", [64, C], FP, isOutput=False)
    y_d = nc.declare_dram_parameter("y", [N, C], FP, isOutput=True)

    with TileContext(nc) as tc:
        with (
            tc.tile_pool(name="const", bufs=1) as constp,
            tc.tile_pool(name="big", bufs=1) as bigp,
            tc.tile_pool(name="exp", bufs=2) as expp,
            tc.tile_pool(name="ytmp", bufs=4) as ytmpp,
            tc.tile_pool(name="psA", bufs=1, space="PSUM") as psA,
            tc.tile_pool(name="psB", bufs=1, space="PSUM") as psB,
            tc.tile_pool(name="psC", bufs=1, space="PSUM") as psC,
            tc.tile_pool(name="psV", bufs=1, space="PSUM") as psV,
            tc.tile_pool(name="psY", bufs=1, space="PSUM") as psY,
        ):
            pools3 = [(psA, "A"), (psB, "B"), (psC, "C")]
            rot = [0]

            def rslab():
                pool, tag = pools3[rot[0] % 3]
                rot[0] += 1
                return pool.tile([P, 1024], FP, tag=tag, name=f"rs{rot[0]}")

            ident = constp.tile([P, P], FP, tag="ident")
            make_identity(nc, ident[:])
            bias0 = constp.tile([P, 1], FP, tag="bias0")
            nc.gpsimd.memset(bias0[:], 0.0)

            # ---- persistent SBUF ----
            xtb = bigp.tile([P, 2, N], BF, tag="xtb")
            qbQ = bigp.tile([64, ITILES, 512], BF, tag="qbQ")
            qbK = bigp.tile([64, NCH, P], BF, tag="qbK")
            vbf = [bigp.tile([P, NCH, 33], BF, tag=f"vbf{h}", name=f"vbf{h}")
                   for h in range(2)]
            wqk = bigp.tile([P, 2, 128], BF, tag="wqk")
            wvb = bigp.tile([P, 2, 64], BF, tag="wvb")
            wo_st = bigp.tile([64, C], FP, tag="wo_st")
            wo_sb = [bigp.tile([32, C], FR, tag=f"wo{h}", name=f"wo{h}")
                     for h in range(2)]
            outd = [bigp.tile([33, N], FR, tag=f"outd{h}", name=f"outd{h}")
                    for h in range(2)]
            rden = bigp.tile([P, 64], FP, tag="rden")
            yacc = bigp.tile([P, NCH, C], BF, tag="yacc")

            # ---- loads ----
            dmaq = [nc.sync, nc.scalar, nc.gpsimd]
            nc.scalar.dma_start(out=wqk[:], in_=wqk_d[:])
            for e in range(8):
                dmaq[e % 3].dma_start(
                    out=xtb[:, :, 512 * e: 512 * (e + 1)],
                    in_=x_d[:, :, 512 * e: 512 * (e + 1)],
                )
            nc.gpsimd.dma_start(out=wvb[:], in_=wv_d[:])
            nc.sync.dma_start(out=wo_st[:], in_=wo_d[:])
            nc.vector.tensor_copy(out=wo_sb[0][:], in_=wo_st[0:32, :])
            nc.vector.tensor_copy(out=wo_sb[1][:], in_=wo_st[32:64, :])
            for h in range(2):
                nc.gpsimd.memset(vbf[h][:, :, 32:33], 1.0)

            # ---- Q/K projections: 8 windows of 512 ----
            # per window: q -> [0:64, cols 0:512], k -> [64:128, cols 512:1024]
            # (separate column regions so each gets its own psum zero-region)
            for w in range(ITILES):
                slab = rslab()
                qv = slab[0:64, 0:512]
                kv = slab[64:128, 0:512]
                for cc in range(2):
                    nc.tensor.matmul(
                        slab[:, 0:512], lhsT=wqk[:, cc, :],
                        rhs=xtb[:, cc, 512 * w: 512 * (w + 1)],
                        start=(cc == 0), stop=(cc == 1),
                    )
                if w % 2 == 0:
                    nc.scalar.activation(qbQ[:, w, :], qv, AF.Copy)
                    nc.vector.tensor_copy(
                        out=qbK[:, 4 * w: 4 * w + 4, :],
                        in_=kv.rearrange("p (c j) -> p c j", j=P),
                    )
                else:
                    nc.vector.tensor_copy(out=qbQ[:, w, :], in_=qv)
                    nc.scalar.activation(
                        qbK[:, 4 * w: 4 * w + 4, :],
                        kv.rearrange("p (c j) -> p c j", j=P),
                        AF.Copy,
                    )

            # ---- V projection (both heads at once) ----
            for half in range(2):
                vslab = rslab()
                c0 = 16 * half
                vv = vslab[:, 0:1024].rearrange("p (c m) -> p c m", m=64)
                for c in range(16):
                    for cc in range(2):
                        nc.tensor.matmul(
                            vv[:, c, :],
                            lhsT=xtb[:, cc, P * (c0 + c): P * (c0 + c + 1)],
                            rhs=wvb[:, cc, :],
                            start=(cc == 0), stop=(cc == 1),
                        )
                nc.vector.tensor_copy(
                    out=vbf[0][:, c0: c0 + 16, 0:32], in_=vv[:, :, 0:32]
                )
                nc.scalar.activation(
                    vbf[1][:, c0: c0 + 16, 0:32], vv[:, :, 32:64], AF.Copy
                )

            # ---- attention ----
            def attention(h, post_it=None):
                for it in range(ITILES):
                    i0 = 512 * it
                    av = psV.tile([P, 512], FP, tag="V")
                    e16 = expp.tile([P, NCH, 512], BF, tag="E")

                    def attnv(c):
                        nc.tensor.matmul(
                            av[0:33, :],
                            lhsT=vbf[h][:, c, :],
                            rhs=e16[:, c, :],
                            start=(c == 0), stop=(c == NCH - 1),
                            skip_group_check=True,
                        )

                    cstart = 0
                    adone = 0
                    hist = [0, 0, 0]  # cstart as of 1/2/3 groups ago
                    for gi, gsz in enumerate(GROUPS):
                        slab = rslab()
                        for r in range(gsz):
                            c = cstart + r
                            nc.tensor.matmul(
                                slab[:, 512 * r: 512 * (r + 1)],
                                lhsT=qbK[32 * h: 32 * h + 32, c, :],
                                rhs=qbQ[32 * h: 32 * h + 32, it, :],
                                start=True, stop=True,
                            )
                        # exp of this slab (whole slabs alternate ACT / DVE)
                        ev = e16[:, cstart: cstart + gsz, :].rearrange("p c n -> p (c n)")
                        sl = slab[:, 0: 512 * gsz]
                        if gi % 2 == 0:
                            nc.scalar.activation(
                                ev, sl, AF.Exp, bias=bias0[:], scale=1.0 / A16,
                            )
                        else:
                            nc.vector.tensor_scalar_add(
                                ev.bitcast(U16), sl, B16
                            )
                        # attn@v lagged three slabs behind
                        while adone < hist[2]:
                            attnv(adone)
                            adone += 1
                        hist = [cstart + gsz, hist[0], hist[1]]
                        cstart += gsz
                    while adone < NCH:
                        attnv(adone)
                        adone += 1
                    if it % 2 == 0:
                        nc.vector.tensor_copy(
                            out=outd[h][:, i0: i0 + 512], in_=av[0:33, :]
                        )
                    else:
                        nc.scalar.activation(
                            outd[h][:, i0: i0 + 512], av[0:33, :], AF.Copy
                        )
                    if post_it is not None:
                        post_it(it)

            def den_recip(h):
                dslab = psY.tile([P, 512], FP, tag="Y")
                for t in range(NCH):
                    nc.tensor.transpose(
                        dslab[:, t: t + 1],
                        outd[h][32:33, P * t: P * (t + 1)].bitcast(FP),
                        ident[32:33, 32:33],
                    )
                nc.vector.reciprocal(
                    out=rden[:, 32 * h: 32 * h + 32], in_=dslab[:, 0:32]
                )

            def y_chunks(it):
                k0 = 4 * it
                # head-0 projection into yacc (rden[0] fully ready)
                for half in range(2):
                    yp = psY.tile([P, 512], FP, tag="Y")
                    for i in range(2):
                        k = k0 + 2 * half + i
                        nc.tensor.matmul(
                            yp[:, C * i: C * (i + 1)],
                            lhsT=outd[0][0:32, P * k: P * (k + 1)],
                            rhs=wo_sb[0][:],
                            start=True, stop=True,
                        )
                    for i in range(2):
                        k = k0 + 2 * half + i
                        nc.scalar.activation(
                            yacc[:, k, :], yp[:, C * i: C * (i + 1)], AF.Copy,
                            scale=rden[:, k: k + 1],
                        )
                # head-1 denominators for this i-tile
                dslab = psY.tile([P, 512], FP, tag="Y")
                for t in range(k0, k0 + 4):
                    nc.tensor.transpose(
                        dslab[:, t - k0: t - k0 + 1],
                        outd[1][32:33, P * t: P * (t + 1)].bitcast(FP),
                        ident[32:33, 32:33],
                    )
                nc.vector.reciprocal(
                    out=rden[:, 32 + k0: 36 + k0], in_=dslab[:, 0:4]
                )
                # head-1 projection + combine + store
                for half in range(2):
                    yb = psY.tile([P, 512], FP, tag="Y")
                    for i in range(2):
                        k = k0 + 2 * half + i
                        nc.tensor.matmul(
                            yb[:, C * i: C * (i + 1)],
                            lhsT=outd[1][0:32, P * k: P * (k + 1)],
                            rhs=wo_sb[1][:],
                            start=True, stop=True,
                        )
                    for i in range(2):
                        k = k0 + 2 * half + i
                        yo = ytmpp.tile([P, C], FP, tag="yo")
                        nc.vector.scalar_tensor_tensor(
                            out=yo[:], in0=yb[:, C * i: C * (i + 1)],
                            scalar=rden[:, 32 + k: 33 + k],
                            in1=yacc[:, k, :], op0=ALU.mult, op1=ALU.add,
                        )
                        dmaq[k % 3].dma_start(
                            out=y_d[P * k: P * (k + 1), :], in_=yo[:]
                        )

            def den0_chunks(it):
                dslab = psY.tile([P, 512], FP, tag="Y")
                for t in range(4 * it, 4 * it + 4):
                    nc.tensor.transpose(
                        dslab[:, t - 4 * it: t - 4 * it + 1],
                        outd[0][32:33, P * t: P * (t + 1)].bitcast(FP),
                        ident[32:33, 32:33],
                    )
                nc.vector.reciprocal(
                    out=rden[:, 4 * it: 4 * it + 4], in_=dslab[:, 0:4]
                )

            attention(0, post_it=den0_chunks)
            attention(1, post_it=y_chunks)

    _split_excess_waits(nc, mybir)
    return nc


def _split_excess_waits(nc, mybir, maxw=1, carrier_cap=1):
    """walrus codegen allows few semaphore waits per engine instruction.

    Tile's scheduler can emit 3-4 on one matmul. Hoist the excess onto
    InstEventSemaphore carriers inserted immediately before the instruction
    on the same engine queue (queue is FIFO, so waiting in the carrier is
    equivalent; no reordering so no deadlock risk).
    """
    skip = {
        "InstEventSemaphore", "InstCall",
        "InstUnconditionalBranch", "InstISA", "InstRegisterMove",
    }
    for f in nc.m.functions:
        for blk in f.blocks:
            idx = 0
            while idx < len(blk.instructions):
                ins = blk.instructions[idx]
                si = getattr(ins, "sync_info", None)
                if (
                    si is not None and si.on_wait and len(si.on_wait) > maxw
                    and type(ins).__name__ not in skip
                ):
                    waits = list(si.on_wait)
                    keep, excess = waits[:maxw], waits[maxw:]
                    n_ins = 0
                    for i in range(0, len(excess), carrier_cap):
                        ev = mybir.InstEventSemaphore(
                            name=nc.get_next_instruction_name(),
                            engine=ins.engine,
                            ins=[], outs=[],
                            sync_info=mybir.SyncInfo(
                                on_wait=excess[i:i + carrier_cap], on_update=[]
                            ),
                        )
                        nc.register_instruction(ev)
                        blk.instructions.insert(idx + n_ins, ev)
                        n_ins += 1
                    ins.sync_info = mybir.SyncInfo(
                        on_wait=keep, on_update=list(si.on_update or [])
                    )
                    idx += n_ins
                idx += 1
    return nc


def get_nc():
    if "nc" not in _CACHED:
        _CACHED["nc"] = _build_nc()
    return _CACHED["nc"]


def make_in_maps(x, w_qkv, w_out):
    """Host-side sharding: core c -> batch c//2, heads (c%2)*2, (c%2)*2+1."""
    import ml_dtypes

    bft = ml_dtypes.bfloat16
    B = x.shape[0]
    xf = np.ascontiguousarray(np.asarray(x, np.float32).reshape(B, N, C))
    wq_all = np.asarray(w_qkv, np.float32)
    wo_all = np.asarray(w_out, np.float32)
    in_maps = []
    for core in range(8):
        b, hp = core // 2, core % 2
        h0 = 2 * hp
        # xtb[p, cc, j] = x[j, cc*128+p]
        xtb = np.ascontiguousarray(
            xf[b].T.reshape(2, P, N).transpose(1, 0, 2)
        ).astype(bft)
        # wqk[p, cc, 0, hh*32+d] = wq[cc*128+p, (h0+hh)*32+d] * F16
        # wqk[p, cc, 1, hh*32+d] = wk[cc*128+p, (h0+hh)*32+d]
        qcols = wq_all[:, h0 * 32: h0 * 32 + 64] * F16          # [256, 64]
        kcols = wq_all[:, 128 + h0 * 32: 128 + h0 * 32 + 64]    # [256, 64]
        wqk = np.concatenate([qcols, kcols], axis=1)             # [256, 128]
        wqk = np.ascontiguousarray(
            wqk.reshape(2, P, 128).transpose(1, 0, 2)
        ).astype(bft)
        vcols = wq_all[:, 256 + h0 * 32: 256 + h0 * 32 + 64]     # [256, 64]
        wvb = np.ascontiguousarray(
            vcols.reshape(2, P, 64).transpose(1, 0, 2)
        ).astype(bft)
        wo = np.concatenate(
            [wo_all[(h0 + hh) * DH: (h0 + hh + 1) * DH, :] for hh in range(2)],
            axis=0,
        )
        in_maps.append({
            "xtb": xtb,
            "wqk": wqk,
            "wvb": wvb,
            "wo": np.ascontiguousarray(wo.astype(np.float32)),
        })
    return in_maps


def kernel(x, w_qkv, w_out, b_out):
    from concourse.bass_utils import run_bass_kernel_spmd

    nc = get_nc()
    in_maps = make_in_maps(x, w_qkv, w_out)
    res = run_bass_kernel_spmd(nc, in_maps, list(range(8))).results
    B, H, W = 4, 64, 64
    y = np.empty((B, N, C), dtype=np.float32)
    for b in range(B):
        y[b] = res[2 * b]["y"] + res[2 * b + 1]["y"]
    y += np.asarray(b_out, dtype=np.float32)
    return y.reshape(B, H, W, C)
